# revision 40
# baseline (speedup 1.0000x reference)
"""GCN encoder (3-layer) as a Bass/Tile kernel on 8 trn2 cores.

Math: PyG GCNConv on a batch of B=4 graphs sharing one edge set.
    deg/norm depend only on edge_index, so the message passing
        agg = segment_sum(norm * (h @ W)[src] -> dst)
    is exactly  A @ (h @ W)  with the dense normalized adjacency
        A[i, j] = sum_{e: dst=i, src=j} norm[e].

Fast path (the actual graph): edge_index is all-pairs + one extra self
loop per node, so deg == N+1 everywhere and A == c * (J + I) with the
scalar c = dinv^2. Then
        agg = c * (S + hw),   S[d'] = sum_i hw[i, d']
and S folds into the activation bias:  per-partition (dim-major layout)
        h' = Relu(c * hwT + (c * S + b)).
S itself comes from one N=1 matmul: S = W.T @ t, t = sum_i hT[:, i].
No adjacency matrix ever touches the device.

General fallback (any other edge_index): build A on the host, run the
dense-matmul formulation (A.T chunks as matmul rhs/lhsT).

Layout: h is kept dim-major (hT: [D=128 partitions, N=512 free]) the
whole way; the final log_softmax transposes z back to node-major with
4 PE transposes so the softmax reduction runs along the free axis.

Sharding: data-parallel over the batch; core c computes graph c % 4 in
full (weights replicated). Outputs gathered from cores 0-3.
"""

import numpy as np

N = 512
B = 4
D = 2  # raw coord dim
H = 128  # embedding dim
L = 3
P = 128
NB = N // P  # node blocks
NUM_CORES = 8

_PROGRAM_CACHE = {}


# f32 bias pack: [b0T | bsT | b0bc]
_CP_B0T = 0
_CP_BST = 1
_CP_B0BC = _CP_BST + L
_CP_COLS = _CP_B0BC + H
# xw pack, 2 partition rows only (real data of w0 and x.T): [w0 | xT]
_XW_W0 = 0
_XW_XT = H
_XW_COLS = _XW_XT + N
# layer-weight pack: ws(l=0..2)
_RP_COLS = L * H


def _patch_act_tables():
    """Point the compiler at an act-table root where the only set holding
    exp/ln is natural_log_exp_and_others. The stock lookup first-matches
    exp -> exp_and_others and ln -> natural_log, so an exp...ln kernel pays
    a ~1.3us mid-kernel ACT_TABLE_LOAD to switch sets; with the combined
    set loaded once at startup there are zero mid-kernel switches."""
    if _PROGRAM_CACHE.get("act_patched"):
        return
    try:
        import glob
        import json
        import os
        import tempfile

        import neuronxcc
        from neuronxcc.driver.jobs.support import FindActInfo

        pkg = os.path.dirname(neuronxcc.__file__)
        src_dir = os.path.join(pkg, "pwp", "pwp_bin_trainium")
        src_json = os.path.join(src_dir, "act_info.json")
        if not os.path.exists(src_json):
            return
        info = json.load(open(src_json))
        names = {s["name"] for s in info.get("act_func_sets", [])}
        if "natural_log_exp_and_others" not in names:
            return
        keep = [s for s in info["act_func_sets"]
                if s["name"] not in ("exp_and_others", "natural_log",
                                     "exp_and_friends")]
        keep.sort(key=lambda s: s["name"] != "natural_log_exp_and_others")
        info["act_func_sets"] = keep
        dst = tempfile.mkdtemp(prefix="act_root_")
        for f in glob.glob(os.path.join(src_dir, "*")):
            base = os.path.basename(f)
            if base != "act_info.json":
                os.symlink(f, os.path.join(dst, base))
        dst_json = os.path.join(dst, "act_info.json")
        json.dump(info, open(dst_json, "w"))

        orig = FindActInfo.findActInfoFile

        def patched(package_dir, arch):
            path = orig(package_dir, arch)
            if os.path.basename(os.path.dirname(path)) == "pwp_bin_trainium":
                return dst_json
            return path

        FindActInfo.findActInfoFile = patched
        from neuronxcc.driver.jobs import WalrusDriver
        if getattr(WalrusDriver, "findActInfoFile", None) is not None:
            WalrusDriver.findActInfoFile = patched
        _PROGRAM_CACHE["act_patched"] = True
    except Exception:
        pass  # fall back to the stock tables (one extra table load)


def _build_structured_program(c_norm: float):
    """A == c_norm * (J + I): no adjacency on device."""
    import concourse.mybir as mybir
    import concourse.tile as tile
    from concourse import bacc
    from concourse.masks import make_identity
    from contextlib import ExitStack

    f32 = mybir.dt.float32
    AF = mybir.ActivationFunctionType
    OP = mybir.AluOpType
    c = float(c_norm)

    nc = bacc.Bacc("TRN2", target_bir_lowering=False, debug=False,
                   num_devices=NUM_CORES)

    # xT/w0 ship as their 2 real rows (5KB, lands in ~0) and are zero-
    # padded to 128 partitions on-device (K<128 matmuls are unsafe; the
    # pad memset covers disjoint partitions so nothing serializes).
    # (fp32r was tried and rejected: DMA-fed fp32r operands truncate to
    # ~tf32 and broke the fp32 error envelope.)
    cpack = nc.dram_tensor("cpack", [P, _CP_COLS], f32,
                           kind="ExternalInput").ap()
    xw2 = nc.dram_tensor("xw2", [32, _XW_COLS], f32,
                         kind="ExternalInput").ap()
    rpack = nc.dram_tensor("rpack", [P, _RP_COLS], f32,
                           kind="ExternalInput").ap()

    upd = nc.dram_tensor("upd", [N, H], f32, kind="ExternalOutput").ap()
    feat = nc.dram_tensor("feat", [N, H], f32, kind="ExternalOutput").ap()

    with tile.TileContext(nc) as tc, ExitStack() as ctx:
        const = ctx.enter_context(tc.tile_pool(name="const", bufs=1))
        hpool = ctx.enter_context(tc.tile_pool(name="hpool", bufs=2))
        work = ctx.enter_context(tc.tile_pool(name="work", bufs=4))
        stat = ctx.enter_context(tc.tile_pool(name="stat", bufs=8))
        psumB = ctx.enter_context(tc.tile_pool(name="psumB", bufs=2, space="PSUM"))
        psumS = ctx.enter_context(tc.tile_pool(name="psumS", bufs=5, space="PSUM"))
        psumKA = ctx.enter_context(tc.tile_pool(name="psumKA", bufs=1, space="PSUM"))

        # first ACT instruction: a throwaway Exp, so walrus loads an
        # exp-capable table set during the DMA window; Relu/Identity are
        # fillers present in every set, so only the final Ln switches sets.
        warm = stat.tile([P, 1], f32, tag="warm")
        nc.vector.memset(warm[:], 0.0)
        nc.scalar.activation(warm[:], warm[:], AF.Exp)

        # spread the input DMA issues across engine DGE queues (each
        # dma_start costs ~600ns of sequencer issue time, serial per engine)
        # and order by first use: xT+w0 (tiny), then weights, then biases.
        xw = const.tile([P, _XW_COLS], f32)
        rp = const.tile([P, _RP_COLS], f32)
        cp = const.tile([P, _CP_COLS], f32)
        nc.vector.memset(xw[32:64, :], 0.0)
        nc.vector.memset(xw[64:96, :], 0.0)
        nc.vector.memset(xw[96:128, :], 0.0)
        nc.sync.dma_start(out=xw[:32, :], in_=xw2[:])
        nc.scalar.dma_start(out=rp[:], in_=rpack[:])
        nc.gpsimd.dma_start(out=cp[:], in_=cpack[:])

        ident = const.tile([P, P], f32)
        make_identity(nc, ident[:])

        # HAM keep-alive: the PE clock (1.2 vs 2.4 GHz) follows a free-
        # running activity window; idle gaps re-throttle it and double every
        # matmul. Filler matmuls during DMA waits / relu gaps keep it warm.
        ka = const.tile([P, H], f32)
        nc.vector.memset(ka[:], 1.0)
        ka_ps = psumKA.tile([P, H], f32, tag="ka")

        def keepalive(n):
            for _ in range(n):
                nc.tensor.matmul(ka_ps[:], ka[:], ka[:], start=True, stop=True)

        b0T_s = cp[:, _CP_B0T:_CP_B0T + 1]
        bsT_s = cp[:, _CP_BST:_CP_BST + L]
        b0bc_s = cp[:, _CP_B0BC:_CP_B0BC + H]
        xT_s = xw[:, _XW_XT:_XW_XT + N]
        w0_s = xw[:, _XW_W0:_XW_W0 + H]

        def ws_l(l):
            return rp[:, l * H:(l + 1) * H]

        # h0T = (x @ W0 + b0).T, dim-major, in two column-halves so the
        # first starts as soon as the first xT half-DMA lands; the single
        # identity applies b0 and its accumulator gives t0 (t must be ready
        # together with hT or the scheduler demotes the S-matmul)
        keepalive(4)
        h0T_ps = psumB.tile([P, N], f32, tag="big")
        nc.tensor.matmul(h0T_ps[:], w0_s, xT_s, start=True, stop=True)
        hT = hpool.tile([P, N], f32, tag="hT")
        t = stat.tile([P, 1], f32, tag="t")
        nc.scalar.activation(hT[:], h0T_ps[:], AF.Identity,
                             bias=b0T_s, accum_out=t[:])
        keepalive(3)

        # h0 node-major blocks (residual + node_feature output), emitted
        # interleaved with the layers: the PE runs them during relu waits
        h0_s = const.tile([P, NB, H], f32)

        def emit_h0_block(b):
            ps = psumS.tile([P, H], f32, tag="blk")
            nc.tensor.matmul(ps[:], xT_s[:, b * P:(b + 1) * P], w0_s,
                             start=True, stop=True)
            nc.vector.tensor_add(out=h0_s[:, b, :], in0=ps[:], in1=b0bc_s)
            nc.sync.dma_start(out=feat[b * P:(b + 1) * P, :], in_=h0_s[:, b, :])

        zT = None
        for l in range(L):
            last = l == L - 1
            # S-matmul first: the bias then computes on DVE while the PE
            # runs the big matmul
            s_ps = psumS.tile([P, H], f32, tag="blk")
            nc.tensor.matmul(s_ps[:, 0:1], ws_l(l), t[:],
                             start=True, stop=True)
            hwT_ps = psumB.tile([P, N], f32, tag="big")
            nc.tensor.matmul(hwT_ps[:], ws_l(l), hT[:],
                             start=True, stop=True)
            bias = stat.tile([P, 1], f32, tag="bias")
            nc.vector.scalar_tensor_tensor(out=bias[:], in0=s_ps[:, 0:1],
                                           scalar=c, in1=bsT_s[:, l:l + 1],
                                           op0=OP.mult, op1=OP.add)
            if not last:
                hT_new = hpool.tile([P, N], f32, tag="hT")
                t = stat.tile([P, 1], f32, tag="t")
                nc.scalar.activation(hT_new[:], hwT_ps[:], AF.Relu,
                                     bias=bias[:, 0:1], scale=c,
                                     accum_out=t[:])
                hT = hT_new
                emit_h0_block(l)  # fills the PE gap under this relu
                keepalive(2)
            else:
                zT = hpool.tile([P, N], f32, tag="zT")
                nc.scalar.activation(zT[:, :N // 2], hwT_ps[:, :N // 2],
                                     AF.Identity, bias=bias[:, 0:1], scale=c)
                nc.scalar.activation(zT[:, N // 2:], hwT_ps[:, N // 2:],
                                     AF.Identity, bias=bias[:, 0:1], scale=c)
                emit_h0_block(2)  # PE gap under the z identity
                emit_h0_block(3)

        # log_softmax along embedding dim + residual, node-major blocks.
        # values are O(+-10), so exp() without max-subtraction is safe.
        z_ps = []
        s_sum = stat.tile([P, NB], f32, tag="ssum")
        for b in range(NB):
            zp = psumS.tile([P, P], f32, tag="blk")
            nc.tensor.transpose(zp[:], zT[:, b * P:(b + 1) * P], ident[:])
            z_ps.append(zp)
        partials = []
        for b in range(NB):
            e = work.tile([P, H], f32, tag="e")
            nc.scalar.activation(e[:], z_ps[b][:], AF.Exp)
            nc.vector.reduce_sum(s_sum[:, b:b + 1], e[:],
                                 axis=mybir.AxisListType.X)
            # overlap with the EXPs: z + h0 now, - lse later
            pt = work.tile([P, H], f32, tag="pt")
            nc.vector.tensor_add(out=pt[:], in0=z_ps[b][:], in1=h0_s[:, b, :])
            partials.append(pt)
        # the combined act set's ln is low-precision (400 ULP budget);
        # one Newton step through exp (same set, no table switch) restores
        # fp32 accuracy: lse = lse0 + s*exp(-lse0) - 1
        lse0 = stat.tile([P, NB], f32, tag="lse0")
        nc.scalar.activation(lse0[:], s_sum[:], AF.Ln)
        u = stat.tile([P, NB], f32, tag="u")
        nc.scalar.activation(u[:], lse0[:], AF.Exp, scale=-1.0)
        us = stat.tile([P, NB], f32, tag="us")
        nc.vector.tensor_mul(out=us[:], in0=u[:], in1=s_sum[:])
        lse = stat.tile([P, NB], f32, tag="lse")
        nc.vector.scalar_tensor_tensor(out=lse[:], in0=us[:], scalar=-1.0,
                                       in1=lse0[:], op0=OP.add, op1=OP.add)
        # consume the keep-alive PSUM so the filler matmuls survive DCE
        nc.vector.tensor_copy(out=warm[:], in_=ka_ps[:, 0:1])

        out_engines = [nc.sync, nc.gpsimd, nc.scalar, nc.sync]
        for b in range(NB):
            o = work.tile([P, H], f32, tag="o")
            nc.vector.tensor_scalar_sub(out=o[:], in0=partials[b][:],
                                        scalar1=lse[:, b:b + 1])
            # spread issue cost across engines so the 4 stores overlap
            out_engines[b].dma_start(out=upd[b * P:(b + 1) * P, :], in_=o[:])

    nc.compile()
    return nc


def _build_general_program():
    """Arbitrary edge_index: dense normalized adjacency as matmuls."""
    import concourse.mybir as mybir
    import concourse.tile as tile
    from concourse import bacc
    from contextlib import ExitStack

    f32 = mybir.dt.float32
    AF = mybir.ActivationFunctionType
    AX = mybir.AxisListType

    nc = bacc.Bacc("TRN2", target_bir_lowering=False, debug=False,
                   num_devices=NUM_CORES)

    xTp = nc.dram_tensor("xTp", [P, N], f32, kind="ExternalInput").ap()
    w0p = nc.dram_tensor("w0p", [P, H], f32, kind="ExternalInput").ap()
    b0T = nc.dram_tensor("b0T", [P, 1], f32, kind="ExternalInput").ap()
    b0bc = nc.dram_tensor("b0bc", [P, H], f32, kind="ExternalInput").ap()
    wsT = nc.dram_tensor("wsT", [P, L, H], f32, kind="ExternalInput").ap()
    bsT = nc.dram_tensor("bsT", [P, L], f32, kind="ExternalInput").ap()
    bs2bc = nc.dram_tensor("bs2bc", [P, H], f32, kind="ExternalInput").ap()
    at = nc.dram_tensor("at", [P, NB, N], f32, kind="ExternalInput").ap()

    upd = nc.dram_tensor("upd", [N, H], f32, kind="ExternalOutput").ap()
    feat = nc.dram_tensor("feat", [N, H], f32, kind="ExternalOutput").ap()

    with tile.TileContext(nc) as tc, ExitStack() as ctx:
        const = ctx.enter_context(tc.tile_pool(name="const", bufs=1))
        hpool = ctx.enter_context(tc.tile_pool(name="hpool", bufs=2))
        work = ctx.enter_context(tc.tile_pool(name="work", bufs=2))
        zpool = ctx.enter_context(tc.tile_pool(name="zpool", bufs=4))
        stat = ctx.enter_context(tc.tile_pool(name="stat", bufs=8))
        psum = ctx.enter_context(tc.tile_pool(name="psum", bufs=3, space="PSUM"))
        psumB = ctx.enter_context(tc.tile_pool(name="psumB", bufs=2, space="PSUM"))

        warm = stat.tile([P, 1], f32, tag="warm")
        nc.vector.memset(warm[:], 1.0)
        nc.scalar.activation(warm[:], warm[:], AF.Ln)

        xT_s = const.tile([P, N], f32)
        nc.sync.dma_start(out=xT_s[:], in_=xTp[:])
        w0_s = const.tile([P, H], f32)
        nc.sync.dma_start(out=w0_s[:], in_=w0p[:])
        ws_s = const.tile([P, L, H], f32)
        nc.sync.dma_start(out=ws_s[:], in_=wsT[:])
        b0T_s = const.tile([P, 1], f32)
        nc.sync.dma_start(out=b0T_s[:], in_=b0T[:])
        bsT_s = const.tile([P, L], f32)
        nc.sync.dma_start(out=bsT_s[:], in_=bsT[:])
        b0bc_s = const.tile([P, H], f32)
        nc.sync.dma_start(out=b0bc_s[:], in_=b0bc[:])
        bs2bc_s = const.tile([P, H], f32)
        nc.sync.dma_start(out=bs2bc_s[:], in_=bs2bc[:])
        at_s = const.tile([P, NB, N], f32)
        nc.sync.dma_start(out=at_s[:], in_=at[:])

        h0T_ps = psumB.tile([P, N], f32, tag="big")
        nc.tensor.matmul(h0T_ps[:], w0_s[:], xT_s[:], start=True, stop=True)
        hT = hpool.tile([P, N], f32, tag="hT")
        nc.vector.tensor_scalar_add(out=hT[:], in0=h0T_ps[:],
                                    scalar1=b0T_s[:, 0:1])

        h0_s = const.tile([P, NB, H], f32)
        for b in range(NB):
            ps = psum.tile([P, H], f32, tag="mm")
            nc.tensor.matmul(ps[:], xT_s[:, b * P:(b + 1) * P], w0_s[:],
                             start=True, stop=True)
            nc.vector.tensor_add(out=h0_s[:, b, :], in0=ps[:], in1=b0bc_s[:])
            nc.sync.dma_start(out=feat[b * P:(b + 1) * P, :], in_=h0_s[:, b, :])

        for l in range(L):
            hw_s = work.tile([P, NB, H], f32, tag="hw")
            for b in range(NB):
                ps = psum.tile([P, H], f32, tag="mm")
                nc.tensor.matmul(ps[:], hT[:, b * P:(b + 1) * P],
                                 ws_s[:, l, :], start=True, stop=True)
                nc.vector.tensor_copy(out=hw_s[:, b, :], in_=ps[:])

            if l < L - 1:
                aggT_ps = psumB.tile([P, N], f32, tag="big")
                for cc in range(NB):
                    nc.tensor.matmul(aggT_ps[:], hw_s[:, cc, :], at_s[:, cc, :],
                                     start=(cc == 0), stop=(cc == NB - 1))
                hT_new = hpool.tile([P, N], f32, tag="hT")
                nc.scalar.activation(hT_new[:], aggT_ps[:], AF.Relu,
                                     bias=bsT_s[:, l:l + 1])
                hT = hT_new
            else:
                z_s = []
                negm_s = []
                s_sum = stat.tile([P, NB], f32, tag="ssum")
                for b in range(NB):
                    agg_ps = psum.tile([P, H], f32, tag="mm")
                    for cc in range(NB):
                        nc.tensor.matmul(agg_ps[:],
                                         at_s[:, cc, b * P:(b + 1) * P],
                                         hw_s[:, cc, :],
                                         start=(cc == 0), stop=(cc == NB - 1))
                    z = zpool.tile([P, H], f32, tag="z")
                    nc.vector.tensor_add(out=z[:], in0=agg_ps[:], in1=bs2bc_s[:])
                    negm = stat.tile([P, 1], f32, tag="negm")
                    nc.vector.reduce_max(negm[:], z[:], axis=AX.X, negate=True)
                    z_s.append(z)
                    negm_s.append(negm)
                for b in range(NB):
                    e = zpool.tile([P, H], f32, tag="e")
                    nc.scalar.activation(e[:], z_s[b][:],
                                         mybir.ActivationFunctionType.Exp,
                                         bias=negm_s[b][:, 0:1],
                                         accum_out=s_sum[:, b:b + 1])
                lse = stat.tile([P, NB], f32, tag="lse")
                nc.scalar.activation(lse[:], s_sum[:],
                                     mybir.ActivationFunctionType.Ln)
                for b in range(NB):
                    tot = stat.tile([P, 1], f32, tag="tot")
                    nc.vector.tensor_sub(out=tot[:], in0=lse[:, b:b + 1],
                                         in1=negm_s[b][:])
                    o = zpool.tile([P, H], f32, tag="o")
                    nc.vector.scalar_tensor_tensor(
                        out=o[:], in0=z_s[b][:], scalar=tot[:, 0:1],
                        in1=h0_s[:, b, :],
                        op0=mybir.AluOpType.subtract, op1=mybir.AluOpType.add)
                    nc.sync.dma_start(out=upd[b * P:(b + 1) * P, :], in_=o[:])

    nc.compile()
    return nc


def _edge_structure(edge_index: np.ndarray):
    """Return c_norm if edge_index is exactly all-pairs + one self loop per
    node (uniform deg = N+1), else None."""
    src = edge_index[0].astype(np.int64)
    dst = edge_index[1].astype(np.int64)
    if src.shape[0] != N * N + N:
        return None
    if src.min() < 0 or src.max() >= N or dst.min() < 0 or dst.max() >= N:
        return None
    counts = np.bincount(src * N + dst, minlength=N * N).reshape(N, N)
    expect = np.ones((N, N), dtype=counts.dtype)
    np.fill_diagonal(expect, 2)
    if not np.array_equal(counts, expect):
        return None
    deg = np.float32(N + 1)
    dinv = (np.float32(1.0) / np.sqrt(deg)).astype(np.float32)
    return float(np.float32(dinv * dinv))


def _build_adjacency(edge_index: np.ndarray) -> np.ndarray:
    """Dense normalized adjacency, transposed: AT[src, dst] (= A.T)."""
    src = edge_index[0].astype(np.int64)
    dst = edge_index[1].astype(np.int64)
    deg = np.bincount(dst, minlength=N).astype(np.float32)
    dinv = np.where(deg > 0, 1.0 / np.sqrt(deg), 0.0).astype(np.float32)
    norm = (dinv[src] * dinv[dst]).astype(np.float32)
    at = np.bincount(src * N + dst, weights=norm.astype(np.float64),
                     minlength=N * N).reshape(N, N)
    return at.astype(np.float32)


def _pad_rows(a: np.ndarray, rows: int) -> np.ndarray:
    out = np.zeros((rows,) + a.shape[1:], dtype=a.dtype)
    out[:a.shape[0]] = a
    return out


def _pack_consts(W0, b0, Ws, bs) -> np.ndarray:
    cp = np.zeros((P, _CP_COLS), dtype=np.float32)
    cp[:, _CP_B0T] = b0
    cp[:, _CP_BST:_CP_BST + L] = bs.T
    cp[:, _CP_B0BC:_CP_B0BC + H] = np.broadcast_to(b0, (P, H))
    return cp


def _pack_xw(x_g, W0) -> np.ndarray:
    xw = np.zeros((32, _XW_COLS), dtype=np.float32)
    xw[:D, _XW_W0:_XW_W0 + H] = W0
    xw[:D, _XW_XT:_XW_XT + N] = x_g.T
    return xw


def _pack_r(Ws) -> np.ndarray:
    return np.ascontiguousarray(
        Ws.transpose(1, 0, 2).reshape(P, _RP_COLS)).astype(np.float32)


def kernel(x, W0, b0, Ws, bs, edge_index):
    from concourse.bass_utils import run_bass_kernel_spmd

    _patch_act_tables()

    x = np.ascontiguousarray(np.asarray(x, dtype=np.float32))
    W0 = np.ascontiguousarray(np.asarray(W0, dtype=np.float32))
    b0 = np.ascontiguousarray(np.asarray(b0, dtype=np.float32))
    Ws = np.ascontiguousarray(np.asarray(Ws, dtype=np.float32))
    bs = np.ascontiguousarray(np.asarray(bs, dtype=np.float32))
    edge_index = np.asarray(edge_index, dtype=np.int32)

    c_norm = _edge_structure(edge_index)
    if c_norm is not None:
        shared = {"cpack": _pack_consts(W0, b0, Ws, bs),
                  "rpack": _pack_r(Ws)}
        key = ("structured", c_norm)
        if key not in _PROGRAM_CACHE:
            _PROGRAM_CACHE[key] = _build_structured_program(c_norm)
        nc = _PROGRAM_CACHE[key]
    else:
        shared = {
            "w0p": _pad_rows(W0, P),
            "b0T": np.ascontiguousarray(b0.reshape(P, 1)),
            "b0bc": np.ascontiguousarray(np.broadcast_to(b0, (P, H))),
            "wsT": np.ascontiguousarray(Ws.transpose(1, 0, 2)),
            "bsT": np.ascontiguousarray(bs.T),
        }
        key = "general"
        if key not in _PROGRAM_CACHE:
            _PROGRAM_CACHE[key] = _build_general_program()
        nc = _PROGRAM_CACHE[key]
        at = _build_adjacency(edge_index)
        shared["at"] = np.ascontiguousarray(
            at.reshape(NB, P, N).transpose(1, 0, 2))
        shared["bs2bc"] = np.ascontiguousarray(
            np.broadcast_to(bs[L - 1], (P, H)))

    in_maps = []
    xws = [_pack_xw(x[g], W0) for g in range(B)] if c_norm is not None \
        else None
    for core in range(NUM_CORES):
        g = core % B
        m = dict(shared)
        if xws is not None:
            m["xw2"] = xws[g]
        else:
            m["xTp"] = _pad_rows(np.ascontiguousarray(x[g].T), P)
        in_maps.append(m)

    res = run_bass_kernel_spmd(nc, in_maps, list(range(NUM_CORES)))
    _PROGRAM_CACHE["last_results"] = res

    upd = np.stack([res.results[g]["upd"] for g in range(B)])
    feat = np.stack([res.results[g]["feat"] for g in range(B)])
    return upd, feat
